# revision 18
# baseline (speedup 1.0000x reference)
"""Trainium2 Bass kernel for StyleGAN2-style 4x4 blur (upfirdn2d, up=down=1,
pad=(2,1)) on x:[8,128,256,256] fp32.

Math: out[i,j] = sum_{p,q in [-2,1]} K[1-p,1-q] * x[i+p, j+q]  (zero-padded),
with K the 4x4 blur kernel. K is rank-1 (outer product), so the conv is
separable: an H-pass with taps from the column factor and a W-pass with taps
from the row factor.

Mapping to hardware: each 1-D conv is a banded-matrix product. Per (b,c)
image (256x256) we run two PSUM-accumulated matmul groups on TensorE using
float32r (relaxed fp32, full-rate at N>=256):

  MM1:  t1[w, h'] = sum_h x[h, w] * BH[h, h']      (H-conv, output transposed)
  MM2:  y[h', w'] = sum_w t1[w, h'] * BW[w, w']    (W-conv, transposes back)

K (contraction) is capped at 128, so each group is 2 accumulating matmuls
over 128-row halves; the 256-wide bands fold the zero padding at the image
borders. ScalarE/VectorE evacuate PSUM->SBUF; HWDGE DMAs stream HBM.

Sharding: batch dim (8) -> one NeuronCore each; channels (128) map to
sequential images per core.
"""

import sys

sys.path.insert(0, "/opt/trn_rl_repo")

import numpy as np

B, C, H, W = 8, 128, 256, 256
KH = KW = 4
N_CORES = 8


def _band_256(taps):
    """Band matrix Bd[k, n] = taps[1 + n - k] for 0 <= 1+n-k < 4, else 0.

    t_out[n] = sum_k Bd[k, n] * x_in[k] is the 1-D conv
    out[n] = sum_{p=-2..1} taps_coeff[p] x[n+p] with taps_coeff[p] = taps[1-p]
    and zero padding (2 leading, 1 trailing) folded in by truncation.
    """
    Bd = np.zeros((256, 256), dtype=np.float64)
    for n in range(256):
        for d in range(4):
            k = n + 1 - d
            if 0 <= k < 256:
                Bd[k, n] = taps[d]
    return Bd


def _factor_kernel(k2):
    """Rank-1 factorization k2 = outer(u, v) (k2 is an outer product)."""
    k2 = np.asarray(k2, dtype=np.float64)
    uu, ss, vv = np.linalg.svd(k2)
    assert ss[1] < 1e-5 * max(ss[0], 1e-30), "blur kernel is not rank-1"
    u = uu[:, 0] * np.sqrt(ss[0])
    v = vv[0] * np.sqrt(ss[0])
    # fix sign so that outer(u, v) ~ k2 with u mostly positive
    if u.sum() < 0:
        u, v = -u, -v
    return u, v


def _make_bands(k2):
    """Returns (bh_sb, bw_sb) as float32 [128, 512] SBUF layouts.

    bh_sb[p, j*256 + n] = BH[2p + j, n] -- input rows interleaved in pairs so
    every DMA partition line is one 2KB-contiguous DRAM chunk (rows 2p, 2p+1).
    bw_sb[p, wb*256 + n] = BW[wb*128 + p, n] -- plain half split (W stays on
    partitions of the intermediate, untouched by the interleave).
    """
    u, v = _factor_kernel(k2)
    # coefficient of x[i+p] is u[1-p] -> band entry BH[k, n] = u[1 + n - k]
    BH = _band_256(u)
    BW = _band_256(v)
    bh_sb = BH.reshape(128, 2, 256).reshape(128, 512).astype(np.float32)
    bw_sb = (
        BW.reshape(2, 128, 256).transpose(1, 0, 2).reshape(128, 512)
    ).astype(np.float32)
    return bh_sb, bw_sb


_NC_CACHE = {}


def _build_nc(n_images, repeats=1, mode="full"):
    import concourse.bacc as bacc
    import concourse.mybir as mybir
    from concourse.tile import TileContext

    f32 = mybir.dt.float32
    f32r = mybir.dt.float32r

    nc = bacc.Bacc("TRN2", target_bir_lowering=False)
    x = nc.dram_tensor("x", (n_images, 256, 256), f32r, kind="ExternalInput")
    bh = nc.dram_tensor("bh", (128, 512), f32r, kind="ExternalInput")
    bw = nc.dram_tensor("bw", (128, 512), f32r, kind="ExternalInput")
    y = nc.dram_tensor("y", (n_images, 256, 256), f32, kind="ExternalOutput")

    # DRAM view: image c, partition p, free = (row-pair member j, col) --
    # partition p holds rows 2p and 2p+1, so each DMA line is 2KB contiguous
    x_v = x.rearrange("c (p j) w -> c p j w", j=2)
    y_v = y.rearrange("c (p j) w -> c p j w", j=2)

    with TileContext(nc) as tc:
        with (
            tc.tile_pool(name="consts", bufs=1) as cpool,
            tc.tile_pool(name="xt", bufs=6) as xpool,
            tc.tile_pool(name="t1", bufs=3) as tpool,
            tc.tile_pool(name="yt", bufs=4) as ypool,
            tc.tile_pool(name="ps1", bufs=2, space="PSUM") as ps1pool,
            tc.tile_pool(name="ps2", bufs=2, space="PSUM") as ps2pool,
        ):
            bh_sb = cpool.tile([128, 512], f32r, tag="bh")
            bw_sb = cpool.tile([128, 512], f32r, tag="bw")
            nc.sync.dma_start(out=bh_sb[:], in_=bh[:])
            nc.sync.dma_start(out=bw_sb[:], in_=bw[:])

            import contextlib

            loop_ctx = (
                tc.For_i(0, repeats, 1) if repeats > 1 else contextlib.nullcontext()
            )
            with loop_ctx:
                _emit_images(nc, range(n_images), x_v, y_v, bh_sb, bw_sb,
                             xpool, tpool, ypool, ps1pool, ps2pool, mode,
                             mybir)

    nc.compile()
    return nc


def _emit_images(nc, images, x_v, y_v, bh_sb, bw_sb, xpool, tpool, ypool,
                 ps1pool, ps2pool, mode, mybir):
    f32 = mybir.dt.float32
    f32r = mybir.dt.float32r
    if True:
        if True:
            for c in images:
                xt = xpool.tile([128, 512], f32r)
                nc.sync.dma_start(
                    out=xt[:].rearrange("p (j w) -> p j w", j=2),
                    in_=x_v[c],
                )
                if mode == "dmaonly":
                    nc.scalar.dma_start(
                        out=y_v[c],
                        in_=xt[:].bitcast(f32).rearrange("p (j w) -> p j w", j=2),
                    )
                    continue

                # MM1: t1[w, h'] = sum_h x[h, w] * BH[h, h']
                # contraction over partitions p (rows 2p+j), accumulated over j
                ps1 = ps1pool.tile([128, 512], f32)
                for wb in range(2):
                    for j in range(2):
                        lhsT = xt[:, j * 256 + wb * 128 : j * 256 + wb * 128 + 128]
                        rhs = bh_sb[:, j * 256 : (j + 1) * 256]
                        nc.tensor.matmul(
                            ps1[:, wb * 256 : (wb + 1) * 256],
                            lhsT,
                            rhs,
                            start=(j == 0),
                            stop=(j == 1),
                        )

                t1 = tpool.tile([128, 512], f32r)
                nc.scalar.copy(out=t1[:], in_=ps1[:])

                # MM2: y[h', w'] = sum_w t1[w, h'] * BW[w, w']
                # lhsT picks h' = 2*h2 + par (strided) so the output tile's
                # partition p holds rows 2p, 2p+1 -> 2KB-contiguous store
                t1v = t1[:].rearrange("p (wb h2 par) -> p wb h2 par", wb=2, par=2)
                ps2 = ps2pool.tile([128, 512], f32)
                for par in range(2):
                    for wb in range(2):
                        lhsT = t1v[:, wb, :, par]
                        rhs = bw_sb[:, wb * 256 : (wb + 1) * 256]
                        nc.tensor.matmul(
                            ps2[:, par * 256 : (par + 1) * 256],
                            lhsT,
                            rhs,
                            start=(wb == 0),
                            stop=(wb == 1),
                        )

                yt = ypool.tile([128, 512], f32)
                nc.vector.tensor_copy(out=yt[:], in_=ps2[:])
                nc.scalar.dma_start(
                    out=y_v[c],
                    in_=yt[:].rearrange("p (j w) -> p j w", j=2),
                )


def _get_nc(n_images, repeats=1, mode="full"):
    key = (n_images, repeats, mode)
    if key not in _NC_CACHE:
        _NC_CACHE[key] = _build_nc(n_images, repeats, mode)
    return _NC_CACHE[key]


def kernel(x, kernel, _trace=False):
    from concourse import bass_utils

    x = np.ascontiguousarray(np.asarray(x), dtype=np.float32)
    k2 = np.asarray(kernel, dtype=np.float32)
    assert x.shape == (B, C, H, W), x.shape
    assert k2.shape == (KH, KW), k2.shape

    bh_sb, bw_sb = _make_bands(k2)

    nc = _get_nc(C)
    in_maps = [{"x": x[b], "bh": bh_sb, "bw": bw_sb} for b in range(B)]
    res = bass_utils.run_bass_kernel_spmd(
        nc, in_maps, core_ids=list(range(N_CORES)), trace=_trace
    )
    out = np.stack([res.results[b]["y"] for b in range(B)], axis=0)
    if _trace:
        return out, res
    return out


# revision 20
# speedup vs baseline: 1.0995x; 1.0995x over previous
"""Trainium2 Bass kernel for StyleGAN2-style 4x4 blur (upfirdn2d, up=down=1,
pad=(2,1)) on x:[8,128,256,256] fp32.

Math: out[i,j] = sum_{p,q in [-2,1]} K[1-p,1-q] * x[i+p, j+q]  (zero-padded),
with K the 4x4 blur kernel. K is rank-1 (outer product), so the conv is
separable: an H-pass with taps from the column factor and a W-pass with taps
from the row factor.

Mapping to hardware: each 1-D conv is a banded-matrix product. Per (b,c)
image (256x256) we run two PSUM-accumulated matmul groups on TensorE using
float32r (relaxed fp32, full-rate at N>=256):

  MM1:  t1[w, h'] = sum_h x[h, w] * BH[h, h']      (H-conv, output transposed)
  MM2:  y[h', w'] = sum_w t1[w, h'] * BW[w, w']    (W-conv, transposes back)

K (contraction) is capped at 128, so each group is 2 accumulating matmuls
over 128-row halves; the 256-wide bands fold the zero padding at the image
borders. ScalarE/VectorE evacuate PSUM->SBUF; HWDGE DMAs stream HBM.

Sharding: batch dim (8) -> one NeuronCore each; channels (128) map to
sequential images per core.
"""

import sys

sys.path.insert(0, "/opt/trn_rl_repo")

import numpy as np

B, C, H, W = 8, 128, 256, 256
KH = KW = 4
N_CORES = 8


def _band_256(taps):
    """Band matrix Bd[k, n] = taps[1 + n - k] for 0 <= 1+n-k < 4, else 0.

    t_out[n] = sum_k Bd[k, n] * x_in[k] is the 1-D conv
    out[n] = sum_{p=-2..1} taps_coeff[p] x[n+p] with taps_coeff[p] = taps[1-p]
    and zero padding (2 leading, 1 trailing) folded in by truncation.
    """
    Bd = np.zeros((256, 256), dtype=np.float64)
    for n in range(256):
        for d in range(4):
            k = n + 1 - d
            if 0 <= k < 256:
                Bd[k, n] = taps[d]
    return Bd


def _factor_kernel(k2):
    """Rank-1 factorization k2 = outer(u, v) (k2 is an outer product)."""
    k2 = np.asarray(k2, dtype=np.float64)
    uu, ss, vv = np.linalg.svd(k2)
    assert ss[1] < 1e-5 * max(ss[0], 1e-30), "blur kernel is not rank-1"
    u = uu[:, 0] * np.sqrt(ss[0])
    v = vv[0] * np.sqrt(ss[0])
    # fix sign so that outer(u, v) ~ k2 with u mostly positive
    if u.sum() < 0:
        u, v = -u, -v
    return u, v


def _make_bands(k2):
    """Returns (bh_sb, bw_sb) as float32 [128, 512] SBUF layouts.

    bh_sb[p, j*256 + n] = BH[2p + j, n] -- input rows interleaved in pairs so
    every DMA partition line is one 2KB-contiguous DRAM chunk (rows 2p, 2p+1).
    bw_sb[p, wb*256 + n] = BW[wb*128 + p, n] -- plain half split (W stays on
    partitions of the intermediate, untouched by the interleave).
    """
    u, v = _factor_kernel(k2)
    # coefficient of x[i+p] is u[1-p] -> band entry BH[k, n] = u[1 + n - k]
    BH = _band_256(u)
    BW = _band_256(v)
    # permute BH's output columns even/odd so MM2 can pick h' = 2i + par with
    # a contiguous 128-column block: column (par*128 + i) holds h' = 2i + par
    perm = np.concatenate([np.arange(0, 256, 2), np.arange(1, 256, 2)])
    BH = BH[:, perm]
    bh_sb = BH.reshape(128, 2, 256).reshape(128, 512).astype(np.float32)
    bw_sb = (
        BW.reshape(2, 128, 256).transpose(1, 0, 2).reshape(128, 512)
    ).astype(np.float32)
    return bh_sb, bw_sb


_NC_CACHE = {}


def _build_nc(n_images, repeats=1, mode="full"):
    import concourse.bacc as bacc
    import concourse.mybir as mybir
    from concourse.tile import TileContext

    f32 = mybir.dt.float32
    f32r = mybir.dt.float32r

    nc = bacc.Bacc("TRN2", target_bir_lowering=False)
    x = nc.dram_tensor("x", (n_images, 256, 256), f32r, kind="ExternalInput")
    bh = nc.dram_tensor("bh", (128, 512), f32r, kind="ExternalInput")
    bw = nc.dram_tensor("bw", (128, 512), f32r, kind="ExternalInput")
    y = nc.dram_tensor("y", (n_images, 256, 256), f32, kind="ExternalOutput")

    # DRAM view: image c, partition p, free = (row-pair member j, col) --
    # partition p holds rows 2p and 2p+1, so each DMA line is 2KB contiguous
    x_v = x.rearrange("c (p j) w -> c p j w", j=2)
    y_v = y.rearrange("c (p j) w -> c p j w", j=2)

    with TileContext(nc) as tc:
        with (
            tc.tile_pool(name="consts", bufs=1) as cpool,
            tc.tile_pool(name="xt", bufs=6) as xpool,
            tc.tile_pool(name="t1", bufs=3) as tpool,
            tc.tile_pool(name="yt", bufs=4) as ypool,
            tc.tile_pool(name="ps1", bufs=2, space="PSUM") as ps1pool,
            tc.tile_pool(name="ps2", bufs=2, space="PSUM") as ps2pool,
        ):
            bh_sb = cpool.tile([128, 512], f32r, tag="bh")
            bw_sb = cpool.tile([128, 512], f32r, tag="bw")
            nc.sync.dma_start(out=bh_sb[:], in_=bh[:])
            nc.sync.dma_start(out=bw_sb[:], in_=bw[:])

            import contextlib

            loop_ctx = (
                tc.For_i(0, repeats, 1) if repeats > 1 else contextlib.nullcontext()
            )
            with loop_ctx:
                _emit_images(nc, range(n_images), x_v, y_v, bh_sb, bw_sb,
                             xpool, tpool, ypool, ps1pool, ps2pool, mode,
                             mybir)

    nc.compile()
    return nc


def _emit_images(nc, images, x_v, y_v, bh_sb, bw_sb, xpool, tpool, ypool,
                 ps1pool, ps2pool, mode, mybir):
    f32 = mybir.dt.float32
    f32r = mybir.dt.float32r
    if True:
        if True:
            for c in images:
                xt = xpool.tile([128, 512], f32r)
                nc.sync.dma_start(
                    out=xt[:].rearrange("p (j w) -> p j w", j=2),
                    in_=x_v[c],
                )
                if mode == "dmaonly":
                    nc.scalar.dma_start(
                        out=y_v[c],
                        in_=xt[:].bitcast(f32).rearrange("p (j w) -> p j w", j=2),
                    )
                    continue

                # MM1: t1[w, h'] = sum_h x[h, w] * BH[h, h']
                # contraction over partitions p (rows 2p+j), accumulated over j
                ps1 = ps1pool.tile([128, 512], f32)
                for wb in range(2):
                    for j in range(2):
                        lhsT = xt[:, j * 256 + wb * 128 : j * 256 + wb * 128 + 128]
                        rhs = bh_sb[:, j * 256 : (j + 1) * 256]
                        nc.tensor.matmul(
                            ps1[:, wb * 256 : (wb + 1) * 256],
                            lhsT,
                            rhs,
                            start=(j == 0),
                            stop=(j == 1),
                        )

                t1 = tpool.tile([128, 512], f32r)
                nc.scalar.copy(out=t1[:], in_=ps1[:])

                # MM2: y[h', w'] = sum_w t1[w, h'] * BW[w, w']
                # BH's columns were even/odd-permuted, so t1's column block
                # (wb, par*128 + i) holds h' = 2i + par: contiguous lhsT gives
                # an output tile whose partition i is rows {2i, 2i+1} ->
                # 2KB-contiguous store
                ps2 = ps2pool.tile([128, 512], f32)
                for par in range(2):
                    for wb in range(2):
                        lhsT = t1[
                            :,
                            wb * 256 + par * 128 : wb * 256 + par * 128 + 128,
                        ]
                        rhs = bw_sb[:, wb * 256 : (wb + 1) * 256]
                        nc.tensor.matmul(
                            ps2[:, par * 256 : (par + 1) * 256],
                            lhsT,
                            rhs,
                            start=(wb == 0),
                            stop=(wb == 1),
                        )

                yt = ypool.tile([128, 512], f32)
                nc.vector.tensor_copy(out=yt[:], in_=ps2[:])
                nc.scalar.dma_start(
                    out=y_v[c],
                    in_=yt[:].rearrange("p (j w) -> p j w", j=2),
                )


def _get_nc(n_images, repeats=1, mode="full"):
    key = (n_images, repeats, mode)
    if key not in _NC_CACHE:
        _NC_CACHE[key] = _build_nc(n_images, repeats, mode)
    return _NC_CACHE[key]


def kernel(x, kernel, _trace=False):
    from concourse import bass_utils

    x = np.ascontiguousarray(np.asarray(x), dtype=np.float32)
    k2 = np.asarray(kernel, dtype=np.float32)
    assert x.shape == (B, C, H, W), x.shape
    assert k2.shape == (KH, KW), k2.shape

    bh_sb, bw_sb = _make_bands(k2)

    nc = _get_nc(C)
    in_maps = [{"x": x[b], "bh": bh_sb, "bw": bw_sb} for b in range(B)]
    res = bass_utils.run_bass_kernel_spmd(
        nc, in_maps, core_ids=list(range(N_CORES)), trace=_trace
    )
    out = np.stack([res.results[b]["y"] for b in range(B)], axis=0)
    if _trace:
        return out, res
    return out


# revision 23
# speedup vs baseline: 1.1899x; 1.0822x over previous
"""Trainium2 Bass kernel for StyleGAN2-style 4x4 blur (upfirdn2d, up=down=1,
pad=(2,1)) on x:[8,128,256,256] fp32.

Math: out[i,j] = sum_{p,q in [-2,1]} K[1-p,1-q] * x[i+p, j+q]  (zero-padded),
with K the 4x4 blur kernel. K is rank-1 (outer product), so the conv is
separable: an H-pass with taps from the column factor and a W-pass with taps
from the row factor.

Mapping to hardware: each 1-D conv is a banded-matrix product. Per (b,c)
image (256x256) we run two PSUM-accumulated matmul groups on TensorE using
float32r (relaxed fp32, full-rate at N>=256):

  MM1:  t1[w, h'] = sum_h x[h, w] * BH[h, h']      (H-conv, output transposed)
  MM2:  y[h', w'] = sum_w t1[w, h'] * BW[w, w']    (W-conv, transposes back)

K (contraction) is capped at 128, so each group is 2 accumulating matmuls
over 128-row halves; the 256-wide bands fold the zero padding at the image
borders. ScalarE/VectorE evacuate PSUM->SBUF; HWDGE DMAs stream HBM.

Sharding: batch dim (8) -> one NeuronCore each; channels (128) map to
sequential images per core.
"""

import os
import sys

sys.path.insert(0, "/opt/trn_rl_repo")

import numpy as np

# DMA layout: "v2" = row-pair interleave (2KB contiguous lines),
# "v1" = half-split (two 1KB chunks per line)
LAYOUT = os.environ.get("BLUR_LAYOUT", "v2")

B, C, H, W = 8, 128, 256, 256
KH = KW = 4
N_CORES = 8


def _band_256(taps):
    """Band matrix Bd[k, n] = taps[1 + n - k] for 0 <= 1+n-k < 4, else 0.

    t_out[n] = sum_k Bd[k, n] * x_in[k] is the 1-D conv
    out[n] = sum_{p=-2..1} taps_coeff[p] x[n+p] with taps_coeff[p] = taps[1-p]
    and zero padding (2 leading, 1 trailing) folded in by truncation.
    """
    Bd = np.zeros((256, 256), dtype=np.float64)
    for n in range(256):
        for d in range(4):
            k = n + 1 - d
            if 0 <= k < 256:
                Bd[k, n] = taps[d]
    return Bd


def _factor_kernel(k2):
    """Rank-1 factorization k2 = outer(u, v) (k2 is an outer product)."""
    k2 = np.asarray(k2, dtype=np.float64)
    uu, ss, vv = np.linalg.svd(k2)
    assert ss[1] < 1e-5 * max(ss[0], 1e-30), "blur kernel is not rank-1"
    u = uu[:, 0] * np.sqrt(ss[0])
    v = vv[0] * np.sqrt(ss[0])
    # fix sign so that outer(u, v) ~ k2 with u mostly positive
    if u.sum() < 0:
        u, v = -u, -v
    return u, v


def _make_bands(k2):
    """Returns (bh_sb, bw_sb) as float32 [128, 512] SBUF layouts.

    bh_sb[p, j*256 + n] = BH[2p + j, n] -- input rows interleaved in pairs so
    every DMA partition line is one 2KB-contiguous DRAM chunk (rows 2p, 2p+1).
    bw_sb[p, wb*256 + n] = BW[wb*128 + p, n] -- plain half split (W stays on
    partitions of the intermediate, untouched by the interleave).
    """
    u, v = _factor_kernel(k2)
    # coefficient of x[i+p] is u[1-p] -> band entry BH[k, n] = u[1 + n - k]
    BH = _band_256(u)
    BW = _band_256(v)
    bw_sb = (
        BW.reshape(2, 128, 256).transpose(1, 0, 2).reshape(128, 512)
    ).astype(np.float32)
    if LAYOUT == "v2":
        # permute BH's output columns even/odd so MM2 can pick h' = 2i + par
        # with a contiguous 128-col block: column (par*128+i) holds h'=2i+par
        perm = np.concatenate([np.arange(0, 256, 2), np.arange(1, 256, 2)])
        BH = BH[:, perm]
        bh_sb = BH.reshape(128, 2, 256).reshape(128, 512).astype(np.float32)
    else:
        bh_sb = (
            BH.reshape(2, 128, 256).transpose(1, 0, 2).reshape(128, 512)
        ).astype(np.float32)
    return bh_sb, bw_sb


_NC_CACHE = {}


def _build_nc(n_images, repeats=1, mode="full"):
    import concourse.bacc as bacc
    import concourse.mybir as mybir
    from concourse.tile import TileContext

    f32 = mybir.dt.float32
    f32r = mybir.dt.float32r

    nc = bacc.Bacc("TRN2", target_bir_lowering=False)
    x = nc.dram_tensor("x", (n_images, 256, 256), f32r, kind="ExternalInput")
    bh = nc.dram_tensor("bh", (128, 512), f32r, kind="ExternalInput")
    bw = nc.dram_tensor("bw", (128, 512), f32r, kind="ExternalInput")
    y = nc.dram_tensor("y", (n_images, 256, 256), f32, kind="ExternalOutput")

    if LAYOUT == "v2":
        # partition p holds rows 2p and 2p+1: each DMA line is 2KB contiguous
        x_v = x.rearrange("c (p j) w -> c p j w", j=2)
        y_v = y.rearrange("c (p j) w -> c p j w", j=2)
    else:
        # partition p holds rows p and 128+p: two 1KB chunks per line
        x_v = x.rearrange("c (j p) w -> c p j w", p=128)
        y_v = y.rearrange("c (j p) w -> c p j w", p=128)

    with TileContext(nc) as tc:
        with (
            tc.tile_pool(name="consts", bufs=1) as cpool,
            tc.tile_pool(name="xt", bufs=6) as xpool,
            tc.tile_pool(name="t1", bufs=3) as tpool,
            tc.tile_pool(name="yt", bufs=4) as ypool,
            tc.tile_pool(name="ps1", bufs=2, space="PSUM") as ps1pool,
            tc.tile_pool(name="ps2", bufs=2, space="PSUM") as ps2pool,
        ):
            bh_sb = cpool.tile([128, 512], f32r, tag="bh")
            bw_sb = cpool.tile([128, 512], f32r, tag="bw")
            nc.sync.dma_start(out=bh_sb[:], in_=bh[:])
            nc.sync.dma_start(out=bw_sb[:], in_=bw[:])

            import contextlib

            loop_ctx = (
                tc.For_i(0, repeats, 1) if repeats > 1 else contextlib.nullcontext()
            )
            with loop_ctx:
                _emit_images(nc, range(n_images), x_v, y_v, bh_sb, bw_sb,
                             xpool, tpool, ypool, ps1pool, ps2pool, mode,
                             mybir)

    nc.compile()
    return nc


def _emit_images(nc, images, x_v, y_v, bh_sb, bw_sb, xpool, tpool, ypool,
                 ps1pool, ps2pool, mode, mybir):
    f32 = mybir.dt.float32
    f32r = mybir.dt.float32r
    if True:
        if True:
            for c in images:
                xt = xpool.tile([128, 512], f32r)
                nc.sync.dma_start(
                    out=xt[:].rearrange("p (j w) -> p j w", j=2),
                    in_=x_v[c],
                )
                if mode == "dmaonly":
                    nc.scalar.dma_start(
                        out=y_v[c],
                        in_=xt[:].bitcast(f32).rearrange("p (j w) -> p j w", j=2),
                    )
                    continue

                # MM1: t1[w, h'] = sum_h x[h, w] * BH[h, h']
                # contraction over partitions p (rows 2p+j), accumulated over j
                ps1 = ps1pool.tile([128, 512], f32)
                for wb in range(2):
                    for j in range(2):
                        lhsT = xt[:, j * 256 + wb * 128 : j * 256 + wb * 128 + 128]
                        rhs = bh_sb[:, j * 256 : (j + 1) * 256]
                        nc.tensor.matmul(
                            ps1[:, wb * 256 : (wb + 1) * 256],
                            lhsT,
                            rhs,
                            start=(j == 0),
                            stop=(j == 1),
                        )

                t1 = tpool.tile([128, 512], f32r)
                nc.scalar.copy(out=t1[:], in_=ps1[:])

                # MM2: y[h', w'] = sum_w t1[w, h'] * BW[w, w']
                # BH's columns were even/odd-permuted, so t1's column block
                # (wb, par*128 + i) holds h' = 2i + par: contiguous lhsT gives
                # an output tile whose partition i is rows {2i, 2i+1} ->
                # 2KB-contiguous store
                ps2 = ps2pool.tile([128, 512], f32)
                for par in range(2):
                    for wb in range(2):
                        lhsT = t1[
                            :,
                            wb * 256 + par * 128 : wb * 256 + par * 128 + 128,
                        ]
                        rhs = bw_sb[:, wb * 256 : (wb + 1) * 256]
                        nc.tensor.matmul(
                            ps2[:, par * 256 : (par + 1) * 256],
                            lhsT,
                            rhs,
                            start=(wb == 0),
                            stop=(wb == 1),
                        )

                yt = ypool.tile([128, 512], f32)
                nc.vector.tensor_copy(out=yt[:], in_=ps2[:])
                nc.scalar.dma_start(
                    out=y_v[c],
                    in_=yt[:].rearrange("p (j w) -> p j w", j=2),
                )


def _get_nc(n_images, repeats=1, mode="full"):
    key = (n_images, repeats, mode)
    if key not in _NC_CACHE:
        _NC_CACHE[key] = _build_nc(n_images, repeats, mode)
    return _NC_CACHE[key]


def kernel(x, kernel, _trace=False):
    from concourse import bass_utils

    x = np.ascontiguousarray(np.asarray(x), dtype=np.float32)
    k2 = np.asarray(kernel, dtype=np.float32)
    assert x.shape == (B, C, H, W), x.shape
    assert k2.shape == (KH, KW), k2.shape

    bh_sb, bw_sb = _make_bands(k2)

    nc = _get_nc(C)
    in_maps = [{"x": x[b], "bh": bh_sb, "bw": bw_sb} for b in range(B)]
    res = bass_utils.run_bass_kernel_spmd(
        nc, in_maps, core_ids=list(range(N_CORES)), trace=_trace
    )
    out = np.stack([res.results[b]["y"] for b in range(B)], axis=0)
    if _trace:
        return out, res
    return out
